# revision 2
# baseline (speedup 1.0000x reference)
"""Trainium2 Bass kernel for the DeepHit-style survival loss.

Math (derived from the reference):
  For each sample i with duration d, event e (u = e>0, st = clip(e-1,0,3)):
    r[k]   = 1 - s[k],  s[k] = sum_c phi[i,c,k]
    lse[k] = log(sum_c e^{phi[i,c,k]} + e^{r[k]})
    loss_i = sum_{k<=d} lse[k] + sum_{k<=d-u} s[k] - u*phi[i,st,d] + (u - d - 1)
  output = mean_i loss_i

Device mapping (per core, 8192 samples = 64 tiles of 128 samples on
partitions; per-octet DMAs, software-pipelined):
  - phi is cast to f16 on the host (tolerance is 2e-2; f16 keeps 10
    mantissa bits) so the DMA moves half the bytes
  - PE: s = sum_c phi_c via f16 identity-matmuls into a per-octet PSUM
    tile; se = sum_c e^phi + e^(1-s) accumulated into a second PSUM
    tile (separate tiles: dependencies are tracked per whole tile, so
    sharing one tile would false-serialize er's read vs emm's write)
  - ACT (per octet): exp over phi (f16 in / f16 out),
    er = e^(1-s) (bf16 out for range, fused affine scale=-1 bias=1),
    lse = ln(se) IN-PLACE
  - DVE: per tile two 128-col scalar_tensor_tensor masked sums with
    accum_out (iota <= d-u over s, iota <= d over lse)
  - PE p-state warmup: dummy matmuls on memset weights keep the PE ramp
    hot through the first real matmul dispatch
  - host: sums partials in f64, adds sum(u - d - 1) and the exact
    gather term -sum(u * phi[i, st, d]) from the f32 input, divides by N

Sharding: pure data parallel over N across 8 cores; the final mean is
reduced on the host from per-sample partials.
"""

import os
import sys
import numpy as np

for _p in ("/opt/trn_rl_repo",):
    if _p not in sys.path:
        sys.path.insert(0, _p)

import concourse.bass as bass
import concourse.bacc as bacc
import concourse.tile as tile
from concourse import mybir
from concourse.bass_utils import run_bass_kernel_spmd
from concourse.instruction_name_ordered_set import InstructionNameOrderedSet

N_CORES = 8
N, QCAUSE, K = 65536, 4, 128
S = N // N_CORES          # samples per core = 8192
T = S // 128              # tiles (128 samples each) per core = 64
NOCT = T // 8             # 8 octets of 8 tiles
ROW = QCAUSE * K          # 512 values per sample

F32 = mybir.dt.float32
F16 = mybir.dt.float16
BF16 = mybir.dt.bfloat16

# PE p-state warmup: dummy identity matmuls keep the PE busy from t~0.3us
# until the first real matmul so the cost model's ramp (full clock only
# after 3us of continuous execution) is hot when real work dispatches.
N_PE_WARM = 42

CHAIN_KEYS = ()

_BUILT = None


def _build_program():
    from contextlib import ExitStack

    nc = bacc.Bacc(
        "TRN2",
        target_bir_lowering=False,
        debug=False,
    )

    phi_d = nc.dram_tensor("phi", [S, ROW], F16, kind="ExternalInput").ap()
    # host-side per-partition tables packed [dsu | dd]:
    #   dsu = d - u, dd = d (masked-sum thresholds)
    cp32_d = nc.dram_tensor("cp32", [128, 2 * T], F32, kind="ExternalInput").ap()
    outM_d = nc.dram_tensor("accM", [128, T], F32, kind="ExternalOutput").ap()
    outL_d = nc.dram_tensor("accL", [128, T], F32, kind="ExternalOutput").ap()

    # Constants baked into the NEFF, packed into one u16 tensor -> one
    # DMA: iota_row (f16) and the f16 identity (weights for both the
    # s-matmul over f16 phi and the e-matmul over f16 exp).
    iota_row = np.tile(np.arange(K, dtype=np.float16), (128, 1))        # [128,128]
    ident_h = np.eye(128, dtype=np.float16)
    cpack16 = np.concatenate(
        [iota_row.view(np.uint16), ident_h.view(np.uint16)], axis=1
    )                                                                   # [128,256]
    cp16_d = nc.inline_tensor(cpack16, name="cp16").ap()

    is_le = mybir.AluOpType.is_le
    mult = mybir.AluOpType.mult
    Exp = mybir.ActivationFunctionType.Exp
    Log = mybir.ActivationFunctionType.Ln

    # Octets 0-3 and NOCT-1 are processed as two 4-tile chunks (own
    # phi/exp tiles): splitting the exp is ACT-cost-neutral and pulls the
    # front of the pipeline several us earlier, shortening the ACT
    # backlog that otherwise sets the drain; the last octet's split
    # shortens the tail chain. er/ln/psum stay per-octet.
    def chunks_of(o):
        return [(0, 4), (4, 4)] if o in (0, 1, 2, 3, NOCT - 1) else [(0, 8)]

    _chain_last = {}

    def chain(key, binst):
        if key not in CHAIN_KEYS:
            return binst
        prev = _chain_last.get(key)
        if prev is not None:
            s = InstructionNameOrderedSet()
            s.add(prev.ins.name)
            binst.ins.add_nosync_dependencies_from(s)
        _chain_last[key] = binst
        return binst

    with tile.TileContext(nc) as tc, ExitStack() as ctx:
        singles = ctx.enter_context(tc.tile_pool(name="singles", bufs=1))
        phip8 = ctx.enter_context(tc.tile_pool(name="phip8", bufs=4))
        phip4 = ctx.enter_context(tc.tile_pool(name="phip4", bufs=8))
        octp = ctx.enter_context(tc.tile_pool(name="octp", bufs=2))
        erp = ctx.enter_context(tc.tile_pool(name="erp", bufs=4))
        junkp = ctx.enter_context(tc.tile_pool(name="junkp", bufs=8))
        psp_s = ctx.enter_context(tc.tile_pool(name="psS", bufs=2, space="PSUM"))
        psp_e = ctx.enter_context(tc.tile_pool(name="psE", bufs=2, space="PSUM"))

        phiC = {}
        expC = {}
        erB = {}
        psS = {}
        psE = {}

        def dma(o, lo, nt):
            pool = phip8 if nt == 8 else phip4
            t = pool.tile([128, nt, ROW], F16, tag=f"phi{nt}")
            src = phi_d[o * 1024 + lo * 128 : o * 1024 + (lo + nt) * 128, :].rearrange(
                "(t p) r -> p t r", t=nt
            )
            chain("SP", nc.sync.dma_start(out=t, in_=src))
            phiC[(o, lo)] = t

        def dma_all(o):
            for lo, nt in chunks_of(o):
                dma(o, lo, nt)

        def exp_(o, lo, nt):
            e = octp.tile([128, nt * ROW], F16, tag=f"exp{nt}")
            chain(
                "ACT",
                nc.scalar.activation(
                    e, phiC[(o, lo)].rearrange("p t r -> p (t r)"), Exp
                ),
            )
            expC[(o, lo)] = e

        def exp_all(o):
            for lo, nt in chunks_of(o):
                exp_(o, lo, nt)

        def smm(o, lo, nt):
            # s = sum_c phi_c: f16 identity-matmuls over the f16 phi tiles
            if lo == 0:
                ps = psp_s.tile([128, 1024], F32, tag="ps", name=f"psS{o}")
                psS[o] = ps
            ps = psS[o]
            for g0 in range(0, nt, 4):
                gn = min(4, nt - g0)
                for c in range(4):
                    rhs = phiC[(o, lo)][:, g0 : g0 + gn, c * K : (c + 1) * K]
                    chain(
                        "PE",
                        nc.tensor.matmul(
                            ps[:, (lo + g0) * K : (lo + g0 + gn) * K],
                            idh,
                            rhs,
                            start=(c == 0),
                            stop=(c == 3),
                        ),
                    )

        def smm_all(o):
            for lo, nt in chunks_of(o):
                smm(o, lo, nt)

        def er_(o):
            e = erp.tile([128, 1024], BF16, tag="er")
            chain(
                "ACT",
                nc.scalar.activation(e, psS[o], Exp, bias=1.0, scale=-1.0),
            )
            erB[o] = e

        def emm_exp(o, lo, nt):
            # se partial = sum_c e^phi via PE accumulation. Separate PSUM
            # tile from s so PE need not wait for er's read (dependencies
            # are tracked per whole tile).
            if lo == 0:
                ps = psp_e.tile([128, 1024], F32, tag="pe", name=f"psE{o}")
                psE[(o, 0)] = ps
            base = lo
            ps = psE[(o, 0)]
            eo = expC[(o, lo)].rearrange("p (t r) -> p t r", t=nt)
            for g0 in range(0, nt, 4):
                gn = min(4, nt - g0)
                for c in range(4):
                    chain(
                        "PE",
                        nc.tensor.matmul(
                            ps[:, (base + g0) * K : (base + g0 + gn) * K],
                            idh,
                            eo[:, g0 : g0 + gn, c * K : (c + 1) * K],
                            start=(c == 0),
                            stop=False,
                        ),
                    )

        def emm_all(o):
            for lo, nt in chunks_of(o):
                emm_exp(o, lo, nt)

        def er_add(o):
            for h_ in range(2):
                chain(
                    "PE",
                    nc.tensor.matmul(
                        psE[(o, 0)][:, h_ * 512 : (h_ + 1) * 512],
                        idh,
                        erB[o][:, h_ * 512 : (h_ + 1) * 512],
                        start=False,
                        stop=True,
                    ),
                )

        def ln_(o):
            ps = psE[(o, 0)]
            chain("ACT", nc.scalar.activation(ps, ps, Log))

        def j12s(o):
            # sum_{k<=d-u} s[k]: mask d-u can be -1 (no terms match)
            for ti in range(8):
                t = o * 8 + ti
                jk = junkp.tile([128, K], F32, tag="j12s")
                chain(
                    "DVE",
                    nc.vector.scalar_tensor_tensor(
                        out=jk,
                        in0=ior,
                        scalar=dsu[:, t : t + 1],
                        in1=psS[o][:, ti * K : (ti + 1) * K],
                        op0=is_le,
                        op1=mult,
                        accum_out=accM[:, t : t + 1],
                    ),
                )

        def j12l(o):
            # sum_{k<=d} lse[k]
            for ti in range(8):
                t = o * 8 + ti
                src_lse = psE[(o, 0)][:, ti * K : (ti + 1) * K]
                jk = junkp.tile([128, K], F32, tag="j12l")
                chain(
                    "DVE",
                    nc.vector.scalar_tensor_tensor(
                        out=jk,
                        in0=ior,
                        scalar=dd[:, t : t + 1],
                        in1=src_lse,
                        op0=is_le,
                        op1=mult,
                        accum_out=accL[:, t : t + 1],
                    ),
                )

        # --- prologue ---
        # PE warmup weights come from a memset (no DMA) so the warmup can
        # start at t~0 while the first phi DMA owns the DMA engines.
        wdm = singles.tile([128, 128], F16)
        chain("DVE", nc.vector.memset(wdm, 1.0))

        dma(0, 0, 4)

        cp32 = singles.tile([128, 2 * T], F32)
        chain("SP", nc.sync.dma_start(out=cp32, in_=cp32_d))

        dma(0, 4, 4)
        dma_all(1)

        # remaining constants in one DMA
        cp16 = singles.tile([128, 2 * K], mybir.dt.uint16)
        chain("SP", nc.sync.dma_start(out=cp16, in_=cp16_d))
        ior = cp16[:, :K].bitcast(F16)
        idh = cp16[:, K:].bitcast(F16)
        dsu = cp32[:, 0:T]
        dd = cp32[:, T : 2 * T]

        accM = singles.tile([128, T], F32)
        accL = singles.tile([128, T], F32)

        # One-time engine reads of the constants: the STT encoding has a
        # tiny sync-wait budget and Tile's wait minimization is per-engine,
        # so the DVE clock must observe the constant-load DMA sems
        # before its first scalar_tensor_tensor.
        warm = singles.tile([128, K], F16)
        chain("DVE", nc.vector.tensor_copy(warm, ior))
        warm2 = singles.tile([128, 2], F32)
        chain("DVE", nc.vector.tensor_copy(warm2[:, 0:1], dsu[:, 0:1]))
        chain("DVE", nc.vector.tensor_copy(warm2[:, 1:2], dd[:, 0:1]))

        # PE p-state warmup
        psd = psp_s.tile([128, 1024], F32, tag="ps")
        for _ in range(N_PE_WARM):
            chain(
                "PE", nc.tensor.matmul(psd[:, 0:128], wdm, wdm, start=True, stop=True)
            )

        dma_all(2)
        dma_all(3)
        for lo, nt in chunks_of(0):
            exp_(0, lo, nt)
            smm(0, lo, nt)
        er_(0)

        # --- software-pipelined steady state ---
        for o in range(NOCT):
            if o + 4 < NOCT:
                dma_all(o + 4)
            nxt = chunks_of(o + 1) if o + 1 < NOCT else []
            if o == 0:
                # ln(0) fills the ACT idle while dma(1) is in flight
                emm_all(0)
                er_add(0)
                ln_(0)
                exp_all(1)
                smm_all(1)
                er_(1)
                j12s(0)
                continue
            if len(nxt) == 2:
                # split next octet: each half gets its own phi/exp tiles
                # so the tail chain is half-sized
                exp_(o + 1, *nxt[0])
                j12l(o - 1)
                emm_all(o)
                er_add(o)
                ln_(o)
                exp_(o + 1, *nxt[1])
                smm_all(o + 1)
                er_(o + 1)
            elif nxt:
                exp_all(o + 1)
                j12l(o - 1)
                emm_all(o)
                er_add(o)
                smm_all(o + 1)
                ln_(o)
                er_(o + 1)
            else:
                j12l(o - 1)
                emm_all(o)
                er_add(o)
                ln_(o)
            j12s(o)
            if o == NOCT - 1:
                chain("SP", nc.sync.dma_start(out=outM_d, in_=accM))
        j12l(NOCT - 1)

        chain("SP", nc.sync.dma_start(out=outL_d, in_=accL))

    # Both Exp and Ln live in the "natural_log_exp_and_others" ACT table
    # set, but the table-load pass picks a set per function greedily and
    # would thrash 2 LoadActFuncSet (~1.3us each) per octet. Restrict the
    # registry (preserving set indices!) so both resolve to the combined
    # set -> a single hoisted load.
    import concourse.bacc as _bacc_mod

    real_get = _bacc_mod.get_activation_tables

    def _only_combined(arch):
        tabs = real_get(arch)
        return {
            name: (fns if name == "natural_log_exp_and_others" else set())
            for name, fns in tabs.items()
        }

    _bacc_mod.get_activation_tables = _only_combined
    try:
        nc.finalize()
    finally:
        _bacc_mod.get_activation_tables = real_get
    return nc


def _get_program():
    global _BUILT
    if _BUILT is None:
        _BUILT = _build_program()
    return _BUILT


def kernel(phi, idx_durations, events):
    phi = np.asarray(phi)
    d = np.asarray(idx_durations).astype(np.int64)
    e = np.asarray(events).astype(np.int64)
    u = (e > 0).astype(np.int64)
    st = np.clip(e - 1, 0, QCAUSE - 1)

    # exact host-side terms: sum(u - d - 1) and the per-sample gather
    # -u * phi[i, st, d] (f64, from the original f32 input)
    phi_flat = np.ascontiguousarray(phi, dtype=np.float32).reshape(N, ROW)
    gath = phi_flat[np.arange(N), st * K + d]
    host_term = float((u - d - 1).sum()) - float((gath * u).astype(np.float64).sum())

    phi16 = phi_flat.astype(np.float16)

    nc = _get_program()

    in_maps = []
    for c in range(N_CORES):
        sl = slice(c * S, (c + 1) * S)
        dc, uc = d[sl], u[sl]
        dsu = (dc - uc).reshape(T, 128).T.astype(np.float32)
        dd = dc.reshape(T, 128).T.astype(np.float32)
        cp32 = np.concatenate([dsu, dd], axis=1)
        in_maps.append(
            {
                "phi": phi16[sl],
                "cp32": np.ascontiguousarray(cp32),
            }
        )

    trace = os.environ.get("BASS_PROFILE") == "1"
    kw = {}
    if trace:
        tmpdir = os.environ.get("BASS_TRACE_DIR") or None
        kw = dict(trace=True, tmpdir=tmpdir)
    res = run_bass_kernel_spmd(nc, in_maps, list(range(N_CORES)), **kw)
    if trace and res.exec_time_ns is not None:
        print(f"HW exec time: {res.exec_time_ns} ns", file=sys.stderr)

    total = 0.0
    for c in range(N_CORES):
        r = res.results[c]
        total += (
            np.asarray(r["accM"], dtype=np.float64).sum()
            + np.asarray(r["accL"], dtype=np.float64).sum()
        )
    total += host_term
    return np.float32(total / N)


if __name__ == "__main__":
    rng = np.random.default_rng(0)
    phi = rng.standard_normal((N, QCAUSE, K), dtype=np.float32)
    d = rng.integers(0, K, size=(N,)).astype(np.int64)
    e = rng.integers(0, QCAUSE + 1, size=(N,)).astype(np.int64)
    print(kernel(phi, d, e))


# revision 5
# speedup vs baseline: 1.5951x; 1.5951x over previous
"""Trainium2 Bass kernel for the DeepHit-style survival loss.

Math (derived from the reference):
  For each sample i with duration d, event e (u = e>0, st = clip(e-1,0,3)):
    r[k]   = 1 - s[k],  s[k] = sum_c phi[i,c,k]
    lse[k] = log(sum_c e^{phi[i,c,k]} + e^{r[k]})
    loss_i = sum_{k<=d} lse[k] + sum_{k<=d-u} s[k] - u*phi[i,st,d] + (u - d - 1)
  output = mean_i loss_i

Key optimization: only columns k <= d_i of sample i ever contribute, so
the host sorts samples by d and packs variable-width octets: octet j of
a core only carries W_j k-columns (W_j = max d in the octet + 1). With
d ~ Uniform[0,128) this halves DMA bytes, exp/ln work and masked-sum
width. Sorted octets are snake-assigned to cores so every core gets the
same width list (one SPMD program) and a balanced load.

Device mapping per core (8 octets of 8 tiles; tile = 128 samples on
partitions; per-octet width W):
  - phi f16 (host cast; tolerance 2e-2), packed [p, tile, cause, k<W],
    DMAed per 4-tile chunk, partition lines contiguous in HBM
  - PE: s = sum_c phi_c via f16 identity matmuls into psS [128, 8W]
    (PSUM f32); se = sum_c e^phi accumulated into psE + er added last
  - ACT per chunk: exp (f16 in/out); per octet: er = e^(1-s) (bf16 for
    range, fused scale=-1 bias=1), lse = ln(psE) in place
  - DVE per tile: two masked sums via scalar_tensor_tensor with
    accum_out (iota <= d-u over psS, iota <= d over psE)
  - PE p-state warmup: dummy matmuls keep the ramp hot
  - host: f64 sum of partials + exact sum(u-d-1) - sum(u*phi[i,st,d])
    from the f32 input

Sharding: pure data parallel over N across 8 cores (samples permuted by
the sort; the mean is permutation invariant).
"""

import os
import sys
import numpy as np

for _p in ("/opt/trn_rl_repo",):
    if _p not in sys.path:
        sys.path.insert(0, _p)

import concourse.bass as bass
import concourse.bacc as bacc
import concourse.tile as tile
from concourse import mybir
from concourse.bass_utils import run_bass_kernel_spmd

N_CORES = 8
N, QCAUSE, K = 65536, 4, 128
S = N // N_CORES          # samples per core = 8192
T = S // 128              # tiles (128 samples each) per core = 64
NOCT = T // 8             # 8 octets of 8 tiles per core

F32 = mybir.dt.float32
F16 = mybir.dt.float16
BF16 = mybir.dt.bfloat16

# PE p-state warmup (cost model: full clock only after 3us of continuous
# execution; any >100ns idle resets the ramp)
N_PE_WARM = 42

_CACHE = {}
_LAST = None


def _build_program(widths):
    """widths: tuple of NOCT per-octet k-widths (each a multiple of 4)."""
    from contextlib import ExitStack

    nc = bacc.Bacc("TRN2", target_bir_lowering=False, debug=False)

    tot = sum(widths)
    # phi packed per partition: for each octet j (width W), [8 tiles x 4
    # causes x W] contiguous; all octets concatenated -> 32*tot values.
    phi_d = nc.dram_tensor("phi", [128, 32 * tot], F16, kind="ExternalInput").ap()
    cp32_d = nc.dram_tensor("cp32", [128, 2 * T], F32, kind="ExternalInput").ap()
    outM_d = nc.dram_tensor("accM", [128, T], F32, kind="ExternalOutput").ap()
    outL_d = nc.dram_tensor("accL", [128, T], F32, kind="ExternalOutput").ap()

    iota_row = np.tile(np.arange(K, dtype=np.float16), (128, 1))        # [128,128]
    ident_h = np.eye(128, dtype=np.float16)
    cpack16 = np.concatenate(
        [iota_row.view(np.uint16), ident_h.view(np.uint16)], axis=1
    )                                                                   # [128,256]
    cp16_d = nc.inline_tensor(cpack16, name="cp16").ap()

    is_le = mybir.AluOpType.is_le
    mult = mybir.AluOpType.mult
    Exp = mybir.ActivationFunctionType.Exp
    Log = mybir.ActivationFunctionType.Ln

    offs = [0]
    for w in widths:
        offs.append(offs[-1] + 32 * w)

    with tile.TileContext(nc) as tc, ExitStack() as ctx:
        singles = ctx.enter_context(tc.tile_pool(name="singles", bufs=1))
        phip = ctx.enter_context(tc.tile_pool(name="phip", bufs=10))
        octp = ctx.enter_context(tc.tile_pool(name="octp", bufs=4))
        erp = ctx.enter_context(tc.tile_pool(name="erp", bufs=3))
        junkp = ctx.enter_context(tc.tile_pool(name="junkp", bufs=8))
        psp_s = ctx.enter_context(tc.tile_pool(name="psS", bufs=2, space="PSUM"))
        psp_e = ctx.enter_context(tc.tile_pool(name="psE", bufs=2, space="PSUM"))

        phiC = {}
        expC = {}
        erB = {}
        psS = {}
        psE = {}

        def dma(o, h):
            # chunk h (0/1) of octet o: 4 tiles x 4 causes x W, packed
            W = widths[o]
            t = phip.tile([128, 4, 4 * W], F16, tag="phi")
            src = phi_d[:, offs[o] + h * 16 * W : offs[o] + (h + 1) * 16 * W]
            nc.sync.dma_start(out=t, in_=src.rearrange("p (t r) -> p t r", t=4))
            phiC[(o, h)] = t

        def dma_all(o):
            dma(o, 0)
            dma(o, 1)

        def exp_(o, h):
            W = widths[o]
            e = octp.tile([128, 16 * W], F16, tag="exp")
            nc.scalar.activation(
                e, phiC[(o, h)].rearrange("p t r -> p (t r)"), Exp
            )
            expC[(o, h)] = e

        # PSUM layout: chunk h (4 tiles x W) lives at column offset 512*h
        # so the two chunks' accumulation groups sit in DIFFERENT PSUM
        # banks -- only one accumulation group may be open per bank, and
        # a group must not cross a bank boundary (4W <= 512 never does).
        def pcol(W, ti):
            return 512 * (ti // 4) + (ti % 4) * W

        def smm(o, h):
            # s = sum_c phi_c for 4 tiles: one matmul per cause
            W = widths[o]
            if h == 0:
                ps = psp_s.tile([128, 1024], F32, tag="ps", name=f"psS{o}")
                psS[o] = ps
            ps = psS[o]
            for c in range(4):
                rhs = phiC[(o, h)][:, :, c * W : (c + 1) * W]
                nc.tensor.matmul(
                    ps[:, 512 * h : 512 * h + 4 * W],
                    idh,
                    rhs,
                    start=(c == 0),
                    stop=(c == 3),
                )

        def er_(o):
            W = widths[o]
            e = erp.tile([128, 8 * W], BF16, tag="er")
            src = psS[o].rearrange("p (h x) -> p h x", h=2)[:, :, : 4 * W]
            nc.scalar.activation(
                e.rearrange("p (h x) -> p h x", h=2), src, Exp, bias=1.0, scale=-1.0
            )
            erB[o] = e

        def emm(o, h):
            W = widths[o]
            if h == 0:
                ps = psp_e.tile([128, 1024], F32, tag="pe", name=f"psE{o}")
                psE[o] = ps
            ps = psE[o]
            eo = expC[(o, h)].rearrange("p (t r) -> p t r", t=4)
            for c in range(4):
                nc.tensor.matmul(
                    ps[:, 512 * h : 512 * h + 4 * W],
                    idh,
                    eo[:, :, c * W : (c + 1) * W],
                    start=(c == 0),
                    stop=False,
                )

        def er_add(o):
            # one matmul per emm chunk group so start/stop regions align
            W = widths[o]
            for h in range(2):
                nc.tensor.matmul(
                    psE[o][:, 512 * h : 512 * h + 4 * W],
                    idh,
                    erB[o][:, 4 * W * h : 4 * W * (h + 1)],
                    start=False,
                    stop=True,
                )

        def ln_(o):
            W = widths[o]
            ps = psE[o].rearrange("p (h x) -> p h x", h=2)[:, :, : 4 * W]
            nc.scalar.activation(ps, ps, Log)

        def j12s(o):
            # sum_{k<=d-u} s[k] (d-u can be -1: no terms match)
            W = widths[o]
            for ti in range(8):
                t = o * 8 + ti
                jk = junkp.tile([128, K], F32, tag="j12s")
                nc.vector.scalar_tensor_tensor(
                    out=jk[:, :W],
                    in0=ior[:, :W],
                    scalar=dsu[:, t : t + 1],
                    in1=psS[o][:, pcol(W, ti) : pcol(W, ti) + W],
                    op0=is_le,
                    op1=mult,
                    accum_out=accM[:, t : t + 1],
                )

        def j12l(o):
            # sum_{k<=d} lse[k]
            W = widths[o]
            for ti in range(8):
                t = o * 8 + ti
                jk = junkp.tile([128, K], F32, tag="j12l")
                nc.vector.scalar_tensor_tensor(
                    out=jk[:, :W],
                    in0=ior[:, :W],
                    scalar=dd[:, t : t + 1],
                    in1=psE[o][:, pcol(W, ti) : pcol(W, ti) + W],
                    op0=is_le,
                    op1=mult,
                    accum_out=accL[:, t : t + 1],
                )

        # --- prologue ---
        wdm = singles.tile([128, 128], F16)
        nc.vector.memset(wdm, 1.0)

        dma(0, 0)

        cp32 = singles.tile([128, 2 * T], F32)
        nc.sync.dma_start(out=cp32, in_=cp32_d)

        dma(0, 1)
        dma_all(1)

        cp16 = singles.tile([128, 2 * K], mybir.dt.uint16)
        nc.sync.dma_start(out=cp16, in_=cp16_d)
        ior = cp16[:, :K].bitcast(F16)
        idh = cp16[:, K:].bitcast(F16)
        dsu = cp32[:, 0:T]
        dd = cp32[:, T : 2 * T]

        accM = singles.tile([128, T], F32)
        accL = singles.tile([128, T], F32)

        # One-time DVE reads of the constants (keeps the per-engine
        # sync-wait minimization from attaching DMA sems to the first STT)
        warm = singles.tile([128, K], F16)
        nc.vector.tensor_copy(warm, ior)
        warm2 = singles.tile([128, 2], F32)
        nc.vector.tensor_copy(warm2[:, 0:1], dsu[:, 0:1])
        nc.vector.tensor_copy(warm2[:, 1:2], dd[:, 0:1])

        # PE p-state warmup
        psd = psp_s.tile([128, 1024], F32, tag="ps")
        for _ in range(N_PE_WARM):
            nc.tensor.matmul(psd[:, 0:128], wdm, wdm, start=True, stop=True)

        dma_all(2)
        dma_all(3)
        exp_(0, 0)
        smm(0, 0)
        exp_(0, 1)
        smm(0, 1)
        er_(0)

        # --- software-pipelined steady state ---
        for o in range(NOCT):
            if o + 4 < NOCT:
                dma_all(o + 4)
            if o > 0:
                j12l(o - 1)
            emm(o, 0)
            emm(o, 1)
            er_add(o)
            ln_(o)
            if o + 1 < NOCT:
                exp_(o + 1, 0)
                smm(o + 1, 0)
                exp_(o + 1, 1)
                smm(o + 1, 1)
                er_(o + 1)
            j12s(o)
            if o == NOCT - 1:
                nc.sync.dma_start(out=outM_d, in_=accM)
        j12l(NOCT - 1)

        nc.sync.dma_start(out=outL_d, in_=accL)

    # Both Exp and Ln live in the "natural_log_exp_and_others" ACT table
    # set; restrict the registry (preserving set indices) so the
    # table-load pass emits a single hoisted load instead of thrashing.
    import concourse.bacc as _bacc_mod

    real_get = _bacc_mod.get_activation_tables

    def _only_combined(arch):
        tabs = real_get(arch)
        return {
            name: (fns if name == "natural_log_exp_and_others" else set())
            for name, fns in tabs.items()
        }

    _bacc_mod.get_activation_tables = _only_combined
    try:
        nc.finalize()
    finally:
        _bacc_mod.get_activation_tables = real_get
    return nc


def _get_program(widths=None):
    global _LAST
    if widths is None:
        assert _LAST is not None, "call kernel() first"
        return _CACHE[_LAST]
    widths = tuple(widths)
    if widths not in _CACHE:
        _CACHE[widths] = _build_program(widths)
    _LAST = widths
    return _CACHE[widths]


def kernel(phi, idx_durations, events):
    phi = np.asarray(phi)
    d = np.asarray(idx_durations).astype(np.int64)
    e = np.asarray(events).astype(np.int64)
    u = (e > 0).astype(np.int64)
    st = np.clip(e - 1, 0, QCAUSE - 1)

    # exact host-side terms: sum(u - d - 1) and the gather
    # -u * phi[i, st, d] (f64, from the original f32 input)
    phi_flat = np.ascontiguousarray(phi, dtype=np.float32).reshape(N, QCAUSE * K)
    gath = phi_flat[np.arange(N), st * K + d]
    host_term = float((u - d - 1).sum()) - float((gath * u).astype(np.float64).sum())

    # sort samples by duration; octet widths come from the sorted order
    perm = np.argsort(d, kind="stable")
    d_s = d[perm]
    u_s = u[perm]
    phi16_s = phi_flat.astype(np.float16)[perm].reshape(N, QCAUSE, K)

    # global sorted octets (1024 samples each); snake-assign to cores
    NG = N // 1024  # 64
    gmax = d_s.reshape(NG, 1024).max(axis=1)
    # per-round width = width of the widest octet in the round (last of
    # the 8), shared by all cores so one SPMD program serves all
    widths = []
    for r in range(NOCT):
        w = int(gmax[8 * r : 8 * r + 8].max()) + 1
        widths.append(max(8, (w + 3) // 4 * 4))
    widths = tuple(widths)

    nc = _get_program(widths)

    in_maps = []
    for c in range(N_CORES):
        bufs = []
        dd_cols = np.empty((128, T), dtype=np.float32)
        dsu_cols = np.empty((128, T), dtype=np.float32)
        for r in range(NOCT):
            g = 8 * r + (c if r % 2 == 0 else 7 - c)
            W = widths[r]
            base = 1024 * g
            block = phi16_s[base : base + 1024].reshape(8, 128, QCAUSE, K)[
                :, :, :, :W
            ]
            bufs.append(
                np.ascontiguousarray(block.transpose(1, 0, 2, 3)).reshape(128, -1)
            )
            dloc = d_s[base : base + 1024].reshape(8, 128).T  # [128, 8 tiles]
            uloc = u_s[base : base + 1024].reshape(8, 128).T
            dd_cols[:, 8 * r : 8 * r + 8] = dloc
            dsu_cols[:, 8 * r : 8 * r + 8] = dloc - uloc
        in_maps.append(
            {
                "phi": np.ascontiguousarray(np.concatenate(bufs, axis=1)),
                "cp32": np.ascontiguousarray(
                    np.concatenate([dsu_cols, dd_cols], axis=1)
                ),
            }
        )

    trace = os.environ.get("BASS_PROFILE") == "1"
    kw = {}
    if trace:
        tmpdir = os.environ.get("BASS_TRACE_DIR") or None
        kw = dict(trace=True, tmpdir=tmpdir)
    res = run_bass_kernel_spmd(nc, in_maps, list(range(N_CORES)), **kw)
    if trace and res.exec_time_ns is not None:
        print(f"HW exec time: {res.exec_time_ns} ns", file=sys.stderr)

    total = 0.0
    for c in range(N_CORES):
        r = res.results[c]
        total += (
            np.asarray(r["accM"], dtype=np.float64).sum()
            + np.asarray(r["accL"], dtype=np.float64).sum()
        )
    total += host_term
    return np.float32(total / N)


if __name__ == "__main__":
    rng = np.random.default_rng(0)
    phi = rng.standard_normal((N, QCAUSE, K), dtype=np.float32)
    d = rng.integers(0, K, size=(N,)).astype(np.int64)
    e = rng.integers(0, QCAUSE + 1, size=(N,)).astype(np.int64)
    print(kernel(phi, d, e))
